# revision 18
# baseline (speedup 1.0000x reference)
# Trainium2 Bass kernel for DirectSoftTreeEnsemble forward pass.
#
# Math (reference):
#   temp = clip(exp(log_temperature), 0.1, 5)
#   logits[b,t,i] = x[b,:] @ split_weights[t,i,:] + split_biases[t,i]      (i: 63 internal nodes)
#   s = sigmoid(logits / temp)
#   mu[b,t,l]     = prod over path of s / (1-s)                            (l: 64 leaves, depth 6)
#   P[t,l,:]      = softmax(leaf_logits[t,l,:] / temp)                     (C=1000 classes)
#   w             = softmax(tree_weights)                                  (T=32 trees)
#   out[b,c]      = sum_{t,l} mu[b,t,l] * w[t] * P[t,l,c]
#
# Strategy: data-parallel over batch (4096 -> 8 cores x 512 rows), tree params
# replicated.  Per core, two big matmuls on the PE array:
#   stage A: [512,1024] @ [1024,2048(ti,padded)]  fp8e4m3 + DoubleRow
#            (2 k-tiles contracted per matmul)
#   stage B: [512,2048(tl)] @ [2048,1000]         bf16
# sigmoid is computed via tanh so ACT needs only one function-table set:
#   2*s = 1 + tanh(z/(2*temp)),  2*(1-s) = 1 - tanh(z/(2*temp))
# The doubling uses the +/-q trick: q = mu*th; left = mu-q; right = mu+q
# (saves the separate (1-th)/(1+th) materialization passes on DVE).
# All row scales are folded into mu^T after the transpose:
#   muT_scaled[tl, b] = mu * w_t*T*1024 / Z_tl
# and the remaining global factor 1/(T*64*1024) = 2^-21 is applied at PSUM
# evacuation (free).  P3 = exp(ll/temp) raw bf16 straight from ACT (the Z
# accumulation rides the exp via accum_out).  Output is stored bf16 and
# upcast on host (halves the output DMA).
# mu^T (stage-B lhsT) is produced by 4 big DMA xbar transposes whose 3D-output
# semantics (out[p,s,b] = in[b, s*128+p]) exactly match the k-tile layout.
# Within each tree's 64 columns the internal nodes are host-permuted so level
# d sits at cols [2^d, 2^(d+1)) in bit-reversed order: every doubling op is
# then a dense step-1 bf16 tensor_tensor (DVE 2x mode), and leaves come out
# in bit-reversed order, absorbed by a host permutation of leaf_logits.
# Leaf logits and stage-A operands travel as fp8e4m3.
#
# Host does only: sharding/layout/dtype prep, the 32-element tree softmax;
# all O(B*...)/O(T*L*C) math runs on device.  Note: the on-device softmax
# skips the max-subtraction (inputs are O(0.1); exp cannot overflow there).

import os

import numpy as np
import ml_dtypes

import concourse.bass as bass
import concourse.mybir as mybir
import concourse.tile as tile
from concourse import bacc
from concourse.bass_utils import run_bass_kernel_spmd

BF16 = mybir.dt.bfloat16
F32 = mybir.dt.float32
FP8 = mybir.dt.float8e4
AF = mybir.ActivationFunctionType
OP = mybir.AluOpType
DR = mybir.MatmulPerfMode.DoubleRow

# Problem shapes (hardcoded per contract)
B, D, C, T, DEPTH = 4096, 1024, 1000, 32, 6
NI = 2**DEPTH - 1          # 63 internal nodes / tree
L = 2**DEPTH               # 64 leaves / tree
NIP = 64                   # padded internal nodes / tree
TIP = T * NIP              # 2048 padded internal total
TL = T * L                 # 2048 leaf rows total
NCORES = 8
BS = B // NCORES           # 512 batch rows / core
MT = BS // 128             # 4 m-tiles / core
KA = D // 128              # 8 k-tiles, stage A
KAP = KA // 2              # 4 k-pairs (DoubleRow), stage A
KB = TL // 128             # 16 k-tiles, stage B
NB_CHUNKS = [(0, 512), (512, C - 512)]  # stage-B n chunks (512, 488)
N_WARMUP_MM = 8
GAMMA = 1.0 / (T * 64 * 1024)   # 2^-21 global evac scale


A_FP8 = True


def _build(has_bias: bool, unit_temp: bool):
    """Build the per-core SPMD Bass program."""
    nc = bacc.Bacc("TRN2", target_bir_lowering=False, debug=False)

    a_dt = FP8 if A_FP8 else BF16
    xT_d = nc.dram_tensor("xT", [D, BS], a_dt, kind="ExternalInput")
    wT_d = nc.dram_tensor("wT", [D, TIP], a_dt, kind="ExternalInput")
    # ll3[p, s, :] = leaf row (s*128 + p); matches the DMA-transpose layout of mu^T
    # fp8: leaf logits are ~N(0, 0.1); quantization washes out in the softmax
    ll_d = nc.dram_tensor("ll", [128, KB, C], FP8, kind="ExternalInput")
    wm_d = nc.dram_tensor("wm", [128, KB], F32, kind="ExternalInput")
    out_d = nc.dram_tensor("out", [BS, C], BF16, kind="ExternalOutput")
    if has_bias:
        bias_d = nc.dram_tensor("biasb", [128, TIP], F32, kind="ExternalInput")
    if not unit_temp:
        lt_d = nc.dram_tensor("lt", [1, 1], F32, kind="ExternalInput")

    with tile.TileContext(nc) as tc:
        consts = tc.alloc_tile_pool(name="consts", bufs=1)
        work = tc.alloc_tile_pool(name="work", bufs=2)
        psp = tc.alloc_tile_pool(name="psp", bufs=4, space="PSUM")

        # ---- temperature scalars -> per-partition [128,1] scale APs ----
        if unit_temp:
            ht_scale = 0.5       # tanh scale: 1/(2*temp)
            et_scale = 1.0       # exp scale: 1/temp
        else:
            ltb = consts.tile([128, 1], F32)
            nc.gpsimd.dma_start(out=ltb, in_=lt_d[:, :].partition_broadcast(128))
            tmp = consts.tile([128, 1], F32)
            nc.scalar.activation(tmp, ltb, AF.Exp)                  # temp
            nc.vector.tensor_scalar(tmp, tmp, 5.0, 0.1, OP.min, OP.max)
            itp = consts.tile([128, 1], F32)
            nc.vector.reciprocal(itp, tmp)                          # 1/temp
            htt = consts.tile([128, 1], F32)
            nc.vector.tensor_scalar_mul(htt, itp, 0.5)              # 1/(2 temp)
            ht_scale = htt[:, :]
            et_scale = itp[:, :]

        # ---- resident inputs, spread over DMA queues; arrival order matters:
        # stage-A operands stream in consumption order on the SP queue, leaf
        # logits on the gpsimd queue so exps start promptly ----
        xTs = consts.tile([128, KA, BS], a_dt)
        wTs = consts.tile([128, KA, TIP], a_dt)
        wm = consts.tile([128, KB], F32)
        ll3 = consts.tile([128, KB, C], FP8)
        xT3 = xT_d[:, :].rearrange("(k p) b -> p k b", p=128)

        def dma_wt(k, eng):
            eng.dma_start(wTs[:, k, :], wT_d[k * 128:(k + 1) * 128, :])

        def dma_xt(j, eng, b0=0, b1=BS):
            eng.dma_start(xTs[:, 2 * j:2 * j + 2, b0:b1],
                          xT3[:, 2 * j:2 * j + 2, b0:b1])

        HB = BS // 2
        # sync (SP/HWDGE) queue carries the stage-A critical path: xT first
        # m-pair interleaved with the WT stream, then the late ll groups and
        # (later, at emission point) the mu transposes.  gpsimd (SWDGE) queue
        # carries the early ll group, wm and the second xT half.  Output
        # stores go on the scalar/vector queues so they never head-of-line
        # block the transposes.
        nc.gpsimd.dma_start(ll3[:, 0:4, :], ll_d[:, 0:4, :])
        nc.gpsimd.dma_start(wm, wm_d[:, :])
        dma_xt(0, nc.sync, 0, HB)   # covers k=0,1
        dma_wt(0, nc.sync)
        dma_wt(1, nc.sync)
        dma_xt(1, nc.sync, 0, HB)   # covers k=2,3
        dma_wt(2, nc.sync)
        dma_wt(3, nc.sync)
        dma_xt(2, nc.sync, 0, HB)   # covers k=4,5
        dma_wt(4, nc.sync)
        dma_wt(5, nc.sync)
        dma_xt(3, nc.sync, 0, HB)   # covers k=6,7
        dma_wt(6, nc.sync)
        dma_wt(7, nc.sync)
        # second x half + remaining ll groups stream behind the stage-A
        # critical path on the sync queue, in consumption order
        for j in range(4):
            dma_xt(j, nc.sync, HB, BS)
        nc.sync.dma_start(ll3[:, 4:8, :], ll_d[:, 4:8, :])
        for g in range(2, 4):
            nc.sync.dma_start(ll3[:, 4 * g:4 * (g + 1), :],
                              ll_d[:, 4 * g:4 * (g + 1), :])
        if has_bias:
            biasb = consts.tile([128, TIP], F32)
            nc.sync.dma_start(biasb, bias_d[:, :])

        P3 = consts.tile([128, KB, C], BF16)
        Z = consts.tile([128, KB], F32)
        muT3 = consts.tile([128, KB, BS], BF16)  # mu^T, lhsT for stage B
        th_t = {}

        # PE warmup: dummy matmuls on a zeroed tile keep the PE busy (and the
        # clock-gate warm) while the first WT/xT chunks are still in flight.
        warm = consts.tile([128, 512], BF16)
        nc.vector.memset(warm, 0.0)
        pwu = psp.tile([128, 512], F32, name="pwu", tag="ps")
        for _ in range(N_WARMUP_MM):
            nc.tensor.matmul(pwu, warm[:, :128], warm[:, :],
                             start=True, stop=True)

        pa_t = {}

        def stage_a_half(m0, m1):
            # k-pair-outer over an m-pair (DoubleRow: 2 k-tiles per matmul):
            # 8 open PSUM accumulation regions track WT chunk arrival, so the
            # PE has runnable matmuls as soon as each k-chunk lands.
            nka = KAP if A_FP8 else KA
            for kk in range(nka):
                for m in (m0, m1):
                    msl = slice(m * 128, (m + 1) * 128)
                    for n in range(4):
                        if kk == 0 and n % 2 == 0:
                            pa_t[(m, n // 2)] = psp.tile(
                                [128, 1024], F32, name=f"pa{m}_{n // 2}",
                                tag="ps")
                        dst = pa_t[(m, n // 2)][:, (n % 2) * 512:(n % 2 + 1) * 512]
                        if A_FP8:
                            nc.tensor.matmul(
                                dst, xTs[:, 2 * kk:2 * kk + 2, msl],
                                wTs[:, 2 * kk:2 * kk + 2, n * 512:(n + 1) * 512],
                                start=(kk == 0), stop=(kk == nka - 1),
                                perf_mode=DR)
                        else:
                            nc.tensor.matmul(
                                dst, xTs[:, kk, msl],
                                wTs[:, kk, n * 512:(n + 1) * 512],
                                start=(kk == 0), stop=(kk == nka - 1))

        def tanh_m(m):
            # two [128, 1024] activations off the 2-bank psum tiles
            th = work.tile([128, TIP], BF16, name=f"th{m}", tag="th")
            th_t[m] = th
            for h in range(2):
                pa = pa_t[(m, h)]
                hsl = slice(h * 1024, (h + 1) * 1024)
                if has_bias:
                    nc.vector.tensor_tensor(pa, pa, biasb[:, hsl], OP.add)
                nc.scalar.activation(th[:, hsl], pa, AF.Tanh, scale=ht_scale)

        def doubling(m):
            # Path-product doubling, all bf16 (DVE 2x mode: step-1, 4B-aligned)
            # with the +/-q trick: q = mu*th, left = mu-q, right = mu+q.
            # Node layout (host-permuted): within each 64-col tree block,
            # level d lives at cols [2^d, 2^(d+1)), in bit-reversed order so
            # the level-d node for LSB-first path j' sits at col 2^d + j'.
            th = th_t[m]
            th3 = th.rearrange("p (t i) -> p t i", t=T)
            muA = work.tile([128, T * 32], BF16, name=f"muA{m}", tag="muA")
            muB = work.tile([128, T * 32], BF16, name=f"muB{m}", tag="muB")
            muQ = work.tile([128, T * 32], BF16, name=f"muQ{m}", tag="muQ")
            mu6 = work.tile([128, TL], BF16, name=f"mu6{m}", tag="mu6")

            def lvl_view(d):
                # mu_d laid out [p, t, 2^d]; odd levels in muA, even in muB
                buf = muA if d % 2 == 1 else muB
                return buf[:, :T * (2 ** d)].rearrange("p (t j) -> p t j", t=T)

            # level 0 (root at col 1): mu1 = [1-th(root), 1+th(root)]
            mu1 = lvl_view(1)
            nc.vector.tensor_scalar(mu1[:, :, 0], th3[:, :, 1], -1.0, 1.0,
                                    OP.mult, OP.add)
            nc.vector.tensor_scalar_add(mu1[:, :, 1], th3[:, :, 1], 1.0)

            for d in range(1, DEPTH):
                lo, hi = 2 ** d, 2 ** (d + 1)
                mu_d = lvl_view(d)
                if d == DEPTH - 1:
                    dst = mu6.rearrange("p (t j) -> p t j", t=T)
                else:
                    dst = lvl_view(d + 1)
                half = 2 ** d
                q = muQ[:, :T * half].rearrange("p (t j) -> p t j", t=T)
                nc.vector.tensor_tensor(q, mu_d, th3[:, :, lo:hi], OP.mult)
                # left children block: mu - q;  right children block: mu + q
                nc.vector.tensor_tensor(dst[:, :, :half], mu_d, q, OP.subtract)
                nc.vector.tensor_tensor(dst[:, :, half:], mu_d, q, OP.add)
            # transpose mu (bf16): muT3[p, s, b] = mu6[b, s*128 + p]
            # On the SP hwdge queue: it is idle once input loads finish.
            msl = slice(m * 128, (m + 1) * 128)
            nc.sync.dma_start_transpose(muT3[:, :, msl], mu6[:, :])

        Zi = consts.tile([128, KB], F32)
        scl = consts.tile([128, KB], F32)

        def leaf_exp(s0, s1, with_scl=True):
            # P3 = exp(ll/temp) raw; Z accumulated for free by ACT.
            for s in range(s0, s1):
                nc.scalar.activation(P3[:, s, :], ll3[:, s, :], AF.Exp,
                                     scale=et_scale, accum_out=Z[:, s:s + 1])
            if with_scl:
                scl_calc(s0, s1)

        def scl_calc(s0, s1):
            # batched: Zi = 1/Z; scl = w*T*1024/Z  (the muT row scale)
            nc.vector.reciprocal(Zi[:, s0:s1], Z[:, s0:s1])
            nc.vector.tensor_tensor(scl[:, s0:s1], Zi[:, s0:s1],
                                    wm[:, s0:s1], OP.mult)

        def scale_mut(eng, s0, s1, b0, b1):
            # muT3[:, s, b0:b1] *= scl[:, s]  (bf16 2x mode, per segment)
            for s in range(s0, s1):
                eng.tensor_scalar_mul(muT3[:, s, b0:b1],
                                      muT3[:, s, b0:b1],
                                      scl[:, s:s + 1])

        pb_t = {}
        outm_t = {}

        def stage_b_pair(m0, m1, k0=0, k1=KB):
            # k-outer over an m-pair so a late P3 segment stalls both
            # m-tiles once instead of each serially.  May be emitted in two
            # k-windows (same open PSUM accumulation group) so the late
            # segments' matmuls sit AFTER their muT scales in the program.
            if k0 == 0:
                for m in (m0, m1):
                    for (c0, cn) in NB_CHUNKS:
                        pb_t[(m, c0)] = psp.tile([128, 1024], F32,
                                                 name=f"pb{m}_{c0}", tag="ps")
            for k in range(k0, k1):
                for m in (m0, m1):
                    msl = slice(m * 128, (m + 1) * 128)
                    for (c0, cn) in NB_CHUNKS:
                        nc.tensor.matmul(
                            pb_t[(m, c0)][:, :cn], muT3[:, k, msl],
                            P3[:, k, c0:c0 + cn],
                            start=(k == 0), stop=(k == KB - 1))

        def evac_store(m):
            # evac with the global 2^-21 scale; bf16 out halves the store
            # DMA.  ACT+scalar-queue for chunk 0, DVE+vector-queue for chunk
            # 1 (keeps the sync queue free for the mu transposes).
            msl = slice(m * 128, (m + 1) * 128)
            outm = work.tile([128, C], BF16, name=f"outm{m}", tag="outm")
            nc.scalar.mul(outm[:, :512], pb_t[(m, 0)][:, :512], GAMMA)
            nc.scalar.dma_start(out_d[msl, :512], outm[:, :512])
            nc.vector.tensor_scalar_mul(outm[:, 512:C],
                                        pb_t[(m, 512)][:, :C - 512], GAMMA)
            nc.scalar.dma_start(out_d[msl, 512:], outm[:, 512:C])

        # Emission order shapes each engine's in-order program.  ACT runs
        # exp0-4 | tanh0 tanh1 | exp5-9 | tanh2 tanh3 | exp10-15 so stage B's
        # per-segment gates land just in time; DVE runs dbl0 dbl1 scl(0..10)
        # dbl2 dbl3 scl-h2; the late h1 scales go to the (idle) gpsimd so
        # they don't trap dbl2/3 behind the exp tail.
        leaf_exp(0, 4)
        stage_a_half(0, 1)
        tanh_m(0)
        tanh_m(1)
        doubling(0)
        doubling(1)
        leaf_exp(4, 10)
        stage_a_half(2, 3)
        scale_mut(nc.vector, 0, 10, 0, 2 * 128)
        # p-state bridge: dummy matmuls keep the PE streak alive between the
        # end of stage A and the first stage-B matmul.  Fresh tile: reusing
        # pwu would clobber a recycled live stage-A accumulator bank.
        pwu2 = psp.tile([128, 512], F32, name="pwu2", tag="ps")
        for _ in range(72):
            nc.tensor.matmul(pwu2, warm[:, :128], warm[:, :],
                             start=True, stop=True)
        stage_b_pair(0, 1, 0, 10)
        tanh_m(2)
        tanh_m(3)
        leaf_exp(10, KB, with_scl=False)
        doubling(2)
        doubling(3)
        scl_calc(10, KB)
        scale_mut(nc.vector, 10, KB, 0, 2 * 128)
        scale_mut(nc.vector, 0, KB, 2 * 128, BS)
        stage_b_pair(0, 1, 10, KB)
        evac_store(0)
        evac_store(1)
        stage_b_pair(2, 3)
        evac_store(2)
        evac_store(3)

        psp.release()
        work.release()
        consts.release()

    nc.compile()
    return nc


_cache = {}


def _get_nc(key):
    if key not in _cache:
        _cache[key] = _build(*key[:2])
    return _cache[key]


def kernel(x, split_weights, split_biases, leaf_logits, tree_weights,
           log_temperature):
    x = np.asarray(x, np.float32)
    split_weights = np.asarray(split_weights, np.float32)
    split_biases = np.asarray(split_biases, np.float32)
    leaf_logits = np.asarray(leaf_logits, np.float32)
    tree_weights = np.asarray(tree_weights, np.float32)
    lt = float(np.asarray(log_temperature, np.float32).reshape(-1)[0])

    has_bias = bool(np.any(split_biases != 0.0))
    unit_temp = (lt == 0.0)
    f8 = ml_dtypes.float8_e4m3 if A_FP8 else ml_dtypes.bfloat16

    # ---- host layout prep ----
    # Node permutation: within each 64-col tree block, col 0 is padding and
    # level d occupies cols [2^d, 2^(d+1)) holding BFS node (2^d-1)+bitrev_d(r)
    # at col 2^d + r; leaves end up in LSB-first path order = bitrev6(BFS).
    def bitrev(v, bits):
        r = 0
        for _ in range(bits):
            r = (r << 1) | (v & 1)
            v >>= 1
        return r

    node_src = np.zeros(NIP, np.int64)  # padded col -> BFS node (col 0 -> pad)
    for d in range(DEPTH):
        for r in range(2 ** d):
            node_src[2 ** d + r] = (2 ** d - 1) + bitrev(r, d)
    leaf_src = np.array([bitrev(j, DEPTH) for j in range(L)], np.int64)

    # W^T [D, TIP]: permuted + padded node columns, fp8
    wpad = np.zeros((T, NIP, D), np.float32)
    wpad[:, 1:, :] = split_weights[:, node_src[1:], :]
    wT = np.ascontiguousarray(wpad.reshape(TIP, D).T.astype(f8))
    # x^T shards [D, BS] per core, fp8
    xT = x.T.astype(f8)
    xT_shards = [np.ascontiguousarray(xT[:, c * BS:(c + 1) * BS])
                 for c in range(NCORES)]
    # leaf logits: bitrev leaf order, then [TL, C] -> [128, KB, C] with
    # ll3[p, s, :] = permuted row s*128+p
    ll_perm = leaf_logits[:, leaf_src, :].reshape(TL, C)
    ll = np.ascontiguousarray(
        ll_perm.reshape(KB, 128, C).transpose(1, 0, 2).astype(f8))
    # tree-weight softmax (32 scalars on host); wm[p, s] = w_t * T * 1024
    # (the 1/Z completes the muT scale; 1/(T*64*1024) lands at evac)
    twf = tree_weights - tree_weights.max()
    w = np.exp(twf) / np.exp(twf).sum()
    wmz = (w * T * 1024.0).astype(np.float32)
    p_idx = np.arange(128)[:, None]
    s_idx = np.arange(KB)[None, :]
    wm = np.ascontiguousarray(wmz[(s_idx * 128 + p_idx) // 64])

    in_map_common = {"wT": wT, "ll": ll, "wm": wm}
    if has_bias:
        bpad = np.zeros((T, NIP), np.float32)
        bpad[:, 1:] = split_biases[:, node_src[1:]]
        in_map_common["biasb"] = np.ascontiguousarray(
            np.broadcast_to(bpad.reshape(1, TIP), (128, TIP)).astype(np.float32))
    if not unit_temp:
        in_map_common["lt"] = np.full((1, 1), lt, np.float32)

    nc = _get_nc((has_bias, unit_temp, A_FP8))
    in_maps = [{"xT": xT_shards[c], **in_map_common} for c in range(NCORES)]
    try:
        res = run_bass_kernel_spmd(nc, in_maps, core_ids=list(range(NCORES)))
    except ModuleNotFoundError:
        # BASS_TRACE set but the axon NTFF hook isn't shipped in this
        # container; retry without tracing.
        os.environ["BASS_NEVER_TRACE"] = "1"
        res = run_bass_kernel_spmd(nc, in_maps, core_ids=list(range(NCORES)))
    global LAST_RESULT
    LAST_RESULT = res
    out = np.concatenate([np.asarray(r["out"]).astype(np.float32)
                          for r in res.results], axis=0)
    return np.ascontiguousarray(out)


LAST_RESULT = None


# revision 19
# speedup vs baseline: 1.2352x; 1.2352x over previous
# Trainium2 Bass kernel for DirectSoftTreeEnsemble forward pass.
#
# Math (reference):
#   temp = clip(exp(log_temperature), 0.1, 5)
#   logits[b,t,i] = x[b,:] @ split_weights[t,i,:] + split_biases[t,i]      (i: 63 internal nodes)
#   s = sigmoid(logits / temp)
#   mu[b,t,l]     = prod over path of s / (1-s)                            (l: 64 leaves, depth 6)
#   P[t,l,:]      = softmax(leaf_logits[t,l,:] / temp)                     (C=1000 classes)
#   w             = softmax(tree_weights)                                  (T=32 trees)
#   out[b,c]      = sum_{t,l} mu[b,t,l] * w[t] * P[t,l,c]
#
# Strategy: data-parallel over batch (4096 -> 8 cores x 512 rows), tree params
# replicated.  Per core, two big matmuls on the PE array:
#   stage A: [512,1024] @ [1024,2048(ti,padded)]  fp8e4m3 + DoubleRow
#            (2 k-tiles contracted per matmul)
#   stage B: [512,2048(tl)] @ [2048,1000]         bf16
# sigmoid is computed via tanh so ACT needs only one function-table set:
#   2*s = 1 + tanh(z/(2*temp)),  2*(1-s) = 1 - tanh(z/(2*temp))
# The doubling uses the +/-q trick: q = mu*th; left = mu-q; right = mu+q
# (saves the separate (1-th)/(1+th) materialization passes on DVE).
# All row scales are folded into mu^T after the transpose:
#   muT_scaled[tl, b] = mu * w_t*T*1024 / Z_tl
# and the remaining global factor 1/(T*64*1024) = 2^-21 is applied at PSUM
# evacuation (free).  P3 = exp(ll/temp) raw bf16 straight from ACT (the Z
# accumulation rides the exp via accum_out).  Output is stored bf16 and
# upcast on host (halves the output DMA).
# mu^T (stage-B lhsT) is produced by 4 big DMA xbar transposes whose 3D-output
# semantics (out[p,s,b] = in[b, s*128+p]) exactly match the k-tile layout.
# Within each tree's 64 columns the internal nodes are host-permuted so level
# d sits at cols [2^d, 2^(d+1)) in bit-reversed order: every doubling op is
# then a dense step-1 bf16 tensor_tensor (DVE 2x mode), and leaves come out
# in bit-reversed order, absorbed by a host permutation of leaf_logits.
# Leaf logits and stage-A operands travel as fp8e4m3.
#
# Host does only: sharding/layout/dtype prep, the 32-element tree softmax;
# all O(B*...)/O(T*L*C) math runs on device.  Note: the on-device softmax
# skips the max-subtraction (inputs are O(0.1); exp cannot overflow there).

import os

import numpy as np
import ml_dtypes

import concourse.bass as bass
import concourse.mybir as mybir
import concourse.tile as tile
from concourse import bacc
from concourse.bass_utils import run_bass_kernel_spmd

BF16 = mybir.dt.bfloat16
F32 = mybir.dt.float32
FP8 = mybir.dt.float8e4
AF = mybir.ActivationFunctionType
OP = mybir.AluOpType
DR = mybir.MatmulPerfMode.DoubleRow

# Problem shapes (hardcoded per contract)
B, D, C, T, DEPTH = 4096, 1024, 1000, 32, 6
NI = 2**DEPTH - 1          # 63 internal nodes / tree
L = 2**DEPTH               # 64 leaves / tree
NIP = 64                   # padded internal nodes / tree
TIP = T * NIP              # 2048 padded internal total
TL = T * L                 # 2048 leaf rows total
NCORES = 8
BS = B // NCORES           # 512 batch rows / core
MT = BS // 128             # 4 m-tiles / core
KA = D // 128              # 8 k-tiles, stage A
KAP = KA // 2              # 4 k-pairs (DoubleRow), stage A
KB = TL // 128             # 16 k-tiles, stage B
NB_CHUNKS = [(0, 512), (512, C - 512)]  # stage-B n chunks (512, 488)
N_WARMUP_MM = 8
GAMMA = 1.0 / (T * 64 * 1024)   # 2^-21 global evac scale


A_FP8 = True


def _build(has_bias: bool, unit_temp: bool):
    """Build the per-core SPMD Bass program."""
    nc = bacc.Bacc("TRN2", target_bir_lowering=False, debug=False)

    a_dt = FP8 if A_FP8 else BF16
    xT_d = nc.dram_tensor("xT", [D, BS], a_dt, kind="ExternalInput")
    wT_d = nc.dram_tensor("wT", [D, TIP], a_dt, kind="ExternalInput")
    # ll3[p, s, :] = leaf row (s*128 + p); matches the DMA-transpose layout of mu^T
    # fp8: leaf logits are ~N(0, 0.1); quantization washes out in the softmax
    ll_d = nc.dram_tensor("ll", [128, KB, C], FP8, kind="ExternalInput")
    wm_d = nc.dram_tensor("wm", [128, KB], F32, kind="ExternalInput")
    out_d = nc.dram_tensor("out", [BS, C], BF16, kind="ExternalOutput")
    if has_bias:
        bias_d = nc.dram_tensor("biasb", [128, TIP], F32, kind="ExternalInput")
    if not unit_temp:
        lt_d = nc.dram_tensor("lt", [1, 1], F32, kind="ExternalInput")

    with tile.TileContext(nc) as tc:
        consts = tc.alloc_tile_pool(name="consts", bufs=1)
        work = tc.alloc_tile_pool(name="work", bufs=2)
        psp = tc.alloc_tile_pool(name="psp", bufs=4, space="PSUM")

        # ---- temperature scalars -> per-partition [128,1] scale APs ----
        if unit_temp:
            ht_scale = 0.5       # tanh scale: 1/(2*temp)
            et_scale = 1.0       # exp scale: 1/temp
        else:
            ltb = consts.tile([128, 1], F32)
            nc.gpsimd.dma_start(out=ltb, in_=lt_d[:, :].partition_broadcast(128))
            tmp = consts.tile([128, 1], F32)
            nc.scalar.activation(tmp, ltb, AF.Exp)                  # temp
            nc.vector.tensor_scalar(tmp, tmp, 5.0, 0.1, OP.min, OP.max)
            itp = consts.tile([128, 1], F32)
            nc.vector.reciprocal(itp, tmp)                          # 1/temp
            htt = consts.tile([128, 1], F32)
            nc.vector.tensor_scalar_mul(htt, itp, 0.5)              # 1/(2 temp)
            ht_scale = htt[:, :]
            et_scale = itp[:, :]

        # ---- resident inputs, spread over DMA queues; arrival order matters:
        # stage-A operands stream in consumption order on the SP queue, leaf
        # logits on the gpsimd queue so exps start promptly ----
        xTs = consts.tile([128, KA, BS], a_dt)
        wTs = consts.tile([128, KA, TIP], a_dt)
        wm = consts.tile([128, KB], F32)
        ll3 = consts.tile([128, KB, C], FP8)
        xT3 = xT_d[:, :].rearrange("(k p) b -> p k b", p=128)

        def dma_wt(k, eng):
            eng.dma_start(wTs[:, k, :], wT_d[k * 128:(k + 1) * 128, :])

        def dma_xt(j, eng, b0=0, b1=BS):
            eng.dma_start(xTs[:, 2 * j:2 * j + 2, b0:b1],
                          xT3[:, 2 * j:2 * j + 2, b0:b1])

        HB = BS // 2
        # sync (SP/HWDGE) queue carries the stage-A critical path: xT first
        # m-pair interleaved with the WT stream, then the late ll groups and
        # (later, at emission point) the mu transposes.  gpsimd (SWDGE) queue
        # carries the early ll group, wm and the second xT half.  Output
        # stores go on the scalar/vector queues so they never head-of-line
        # block the transposes.
        nc.gpsimd.dma_start(ll3[:, 0:4, :], ll_d[:, 0:4, :])
        nc.gpsimd.dma_start(wm, wm_d[:, :])
        dma_xt(0, nc.sync, 0, HB)   # covers k=0,1
        dma_wt(0, nc.sync)
        dma_wt(1, nc.sync)
        dma_xt(1, nc.sync, 0, HB)   # covers k=2,3
        dma_wt(2, nc.sync)
        dma_wt(3, nc.sync)
        dma_xt(2, nc.sync, 0, HB)   # covers k=4,5
        dma_wt(4, nc.sync)
        dma_wt(5, nc.sync)
        dma_xt(3, nc.sync, 0, HB)   # covers k=6,7
        dma_wt(6, nc.sync)
        dma_wt(7, nc.sync)
        # second x half + remaining ll groups stream behind the stage-A
        # critical path on the sync queue, in consumption order
        for j in range(4):
            dma_xt(j, nc.sync, HB, BS)
        nc.sync.dma_start(ll3[:, 4:8, :], ll_d[:, 4:8, :])
        for g in range(2, 4):
            nc.sync.dma_start(ll3[:, 4 * g:4 * (g + 1), :],
                              ll_d[:, 4 * g:4 * (g + 1), :])
        if has_bias:
            biasb = consts.tile([128, TIP], F32)
            nc.sync.dma_start(biasb, bias_d[:, :])

        P3 = consts.tile([128, KB, C], BF16)
        Z = consts.tile([128, KB], F32)
        muT3 = consts.tile([128, KB, BS], BF16)  # mu^T, lhsT for stage B
        th_t = {}

        # PE warmup: dummy matmuls on a zeroed tile keep the PE busy (and the
        # clock-gate warm) while the first WT/xT chunks are still in flight.
        warm = consts.tile([128, 512], BF16)
        nc.vector.memset(warm, 0.0)
        pwu = psp.tile([128, 512], F32, name="pwu", tag="ps")
        for _ in range(N_WARMUP_MM):
            nc.tensor.matmul(pwu, warm[:, :128], warm[:, :],
                             start=True, stop=True)

        pa_t = {}

        def stage_a_half(m0, m1):
            # k-pair-outer over an m-pair (DoubleRow: 2 k-tiles per matmul):
            # 8 open PSUM accumulation regions track WT chunk arrival, so the
            # PE has runnable matmuls as soon as each k-chunk lands.
            nka = KAP if A_FP8 else KA
            for kk in range(nka):
                for m in (m0, m1):
                    msl = slice(m * 128, (m + 1) * 128)
                    for n in range(4):
                        if kk == 0 and n % 2 == 0:
                            pa_t[(m, n // 2)] = psp.tile(
                                [128, 1024], F32, name=f"pa{m}_{n // 2}",
                                tag="ps")
                        dst = pa_t[(m, n // 2)][:, (n % 2) * 512:(n % 2 + 1) * 512]
                        if A_FP8:
                            nc.tensor.matmul(
                                dst, xTs[:, 2 * kk:2 * kk + 2, msl],
                                wTs[:, 2 * kk:2 * kk + 2, n * 512:(n + 1) * 512],
                                start=(kk == 0), stop=(kk == nka - 1),
                                perf_mode=DR)
                        else:
                            nc.tensor.matmul(
                                dst, xTs[:, kk, msl],
                                wTs[:, kk, n * 512:(n + 1) * 512],
                                start=(kk == 0), stop=(kk == nka - 1))

        def tanh_m(m):
            # two [128, 1024] activations off the 2-bank psum tiles
            th = work.tile([128, TIP], BF16, name=f"th{m}", tag="th")
            th_t[m] = th
            for h in range(2):
                pa = pa_t[(m, h)]
                hsl = slice(h * 1024, (h + 1) * 1024)
                if has_bias:
                    nc.vector.tensor_tensor(pa, pa, biasb[:, hsl], OP.add)
                nc.scalar.activation(th[:, hsl], pa, AF.Tanh, scale=ht_scale)

        def doubling(m):
            # Path-product doubling, all bf16 (DVE 2x mode: step-1, 4B-aligned)
            # with the +/-q trick: q = mu*th, left = mu-q, right = mu+q.
            # Node layout (host-permuted): within each 64-col tree block,
            # level d lives at cols [2^d, 2^(d+1)), in bit-reversed order so
            # the level-d node for LSB-first path j' sits at col 2^d + j'.
            th = th_t[m]
            th3 = th.rearrange("p (t i) -> p t i", t=T)
            muA = work.tile([128, T * 32], BF16, name=f"muA{m}", tag="muA")
            muB = work.tile([128, T * 32], BF16, name=f"muB{m}", tag="muB")
            muQ = work.tile([128, T * 32], BF16, name=f"muQ{m}", tag="muQ")
            mu6 = work.tile([128, TL], BF16, name=f"mu6{m}", tag="mu6")

            def lvl_view(d):
                # mu_d laid out [p, t, 2^d]; odd levels in muA, even in muB
                buf = muA if d % 2 == 1 else muB
                return buf[:, :T * (2 ** d)].rearrange("p (t j) -> p t j", t=T)

            # level 0 (root at col 1): mu1 = [1-th(root), 1+th(root)]
            mu1 = lvl_view(1)
            nc.vector.tensor_scalar(mu1[:, :, 0], th3[:, :, 1], -1.0, 1.0,
                                    OP.mult, OP.add)
            nc.vector.tensor_scalar_add(mu1[:, :, 1], th3[:, :, 1], 1.0)

            for d in range(1, DEPTH):
                lo, hi = 2 ** d, 2 ** (d + 1)
                mu_d = lvl_view(d)
                if d == DEPTH - 1:
                    dst = mu6.rearrange("p (t j) -> p t j", t=T)
                else:
                    dst = lvl_view(d + 1)
                half = 2 ** d
                q = muQ[:, :T * half].rearrange("p (t j) -> p t j", t=T)
                nc.vector.tensor_tensor(q, mu_d, th3[:, :, lo:hi], OP.mult)
                # left children block: mu - q;  right children block: mu + q
                nc.vector.tensor_tensor(dst[:, :, :half], mu_d, q, OP.subtract)
                nc.vector.tensor_tensor(dst[:, :, half:], mu_d, q, OP.add)
            # transpose mu (bf16): muT3[p, s, b] = mu6[b, s*128 + p]
            # On the SP hwdge queue: it is idle once input loads finish.
            msl = slice(m * 128, (m + 1) * 128)
            nc.sync.dma_start_transpose(muT3[:, :, msl], mu6[:, :])

        Zi = consts.tile([128, KB], F32)
        scl = consts.tile([128, KB], F32)

        def leaf_exp(s0, s1, with_scl=True):
            # P3 = exp(ll/temp) raw; Z accumulated for free by ACT.
            for s in range(s0, s1):
                nc.scalar.activation(P3[:, s, :], ll3[:, s, :], AF.Exp,
                                     scale=et_scale, accum_out=Z[:, s:s + 1])
            if with_scl:
                scl_calc(s0, s1)

        def scl_calc(s0, s1):
            # batched: Zi = 1/Z; scl = w*T*1024/Z  (the muT row scale)
            nc.vector.reciprocal(Zi[:, s0:s1], Z[:, s0:s1])
            nc.vector.tensor_tensor(scl[:, s0:s1], Zi[:, s0:s1],
                                    wm[:, s0:s1], OP.mult)

        def scl_scale_per_s(s0, s1, b0, b1):
            # per-segment scl + muT scale so segment s unblocks as soon as
            # its own exp lands (a batched recip would wait for the last one)
            for s in range(s0, s1):
                scl_calc(s, s + 1)
                nc.vector.tensor_scalar_mul(muT3[:, s, b0:b1],
                                            muT3[:, s, b0:b1],
                                            scl[:, s:s + 1])

        def scale_mut(eng, s0, s1, b0, b1):
            # muT3[:, s, b0:b1] *= scl[:, s]  (bf16 2x mode, per segment)
            for s in range(s0, s1):
                eng.tensor_scalar_mul(muT3[:, s, b0:b1],
                                      muT3[:, s, b0:b1],
                                      scl[:, s:s + 1])

        pb_t = {}
        outm_t = {}

        def stage_b_win(ms, k0, k1):
            # One [128,1024] PSUM tile per m with two accumulation regions
            # ([0:512] and [512:1000]) so all four m-tiles' stage-B PSUMs
            # coexist in 8 banks; emitted in k-windows so each matmul sits
            # after its muT segment's scale in the program.
            for k in range(k0, k1):
                for m in ms:
                    msl = slice(m * 128, (m + 1) * 128)
                    for (c0, cn) in NB_CHUNKS:
                        nc.tensor.matmul(
                            pb_t[m][:, c0:c0 + cn], muT3[:, k, msl],
                            P3[:, k, c0:c0 + cn],
                            start=(k == 0), stop=(k == KB - 1))

        def evac_store(m):
            # evac with the global 2^-21 scale; bf16 out halves the store
            # DMA.  ACT+scalar-queue for chunk 0, DVE+vector-queue for chunk
            # 1 (keeps the sync queue free for the mu transposes).
            msl = slice(m * 128, (m + 1) * 128)
            outm = work.tile([128, C], BF16, name=f"outm{m}", tag="outm")
            nc.scalar.mul(outm[:, :512], pb_t[m][:, :512], GAMMA)
            nc.scalar.dma_start(out_d[msl, :512], outm[:, :512])
            nc.vector.tensor_scalar_mul(outm[:, 512:C],
                                        pb_t[m][:, 512:C], GAMMA)
            nc.scalar.dma_start(out_d[msl, 512:], outm[:, 512:C])

        # Emission order shapes each engine's in-order program.
        # ACT: exp0-3 | tanh0-3 | exp4-9 | exp10-15 | evacs.
        # DVE: scl(0:4) | dbl0 dbl1 | sclh1(0:4) | dbl2 dbl3 | per-s scl+scale
        #      (4:10) | sclh2(0:10) | scl+scales(10:16) | evacs.
        # PE:  warm | A1 | A2 | bridge | B01 k0-9 | B23 k0-9 | B01 k10-15 |
        #      B23 k10-15  (k-windows sit after their scales in the program).
        leaf_exp(0, 4)
        stage_a_half(0, 1)
        tanh_m(0)
        tanh_m(1)
        doubling(0)
        doubling(1)
        stage_a_half(2, 3)
        tanh_m(2)
        tanh_m(3)
        leaf_exp(4, 10, with_scl=False)
        scale_mut(nc.vector, 0, 4, 0, 2 * 128)
        doubling(2)
        doubling(3)
        for m in range(MT):
            pb_t[m] = psp.tile([128, 1024], F32, name=f"pb{m}", tag="ps")
        # p-state bridge: complete dummy groups into pb0's chunk-0 region;
        # the real k0 (start=True) resets the bank, so they are discarded.
        for _ in range(12):
            nc.tensor.matmul(pb_t[0][:, :512], warm[:, :128], warm[:, :],
                             start=True, stop=True)
        stage_b_win((0, 1), 0, 4)
        scl_scale_per_s(4, 10, 0, 2 * 128)
        stage_b_win((0, 1), 4, 10)
        scale_mut(nc.vector, 0, 10, 2 * 128, BS)
        stage_b_win((2, 3), 0, 10)
        leaf_exp(10, KB, with_scl=False)
        scl_calc(10, KB)
        scale_mut(nc.vector, 10, KB, 0, 2 * 128)
        scale_mut(nc.vector, 10, KB, 2 * 128, BS)
        stage_b_win((0, 1), 10, KB)
        stage_b_win((2, 3), 10, KB)
        evac_store(0)
        evac_store(1)
        evac_store(2)
        evac_store(3)

        psp.release()
        work.release()
        consts.release()

    nc.compile()
    return nc


_cache = {}


def _get_nc(key):
    if key not in _cache:
        _cache[key] = _build(*key[:2])
    return _cache[key]


def kernel(x, split_weights, split_biases, leaf_logits, tree_weights,
           log_temperature):
    x = np.asarray(x, np.float32)
    split_weights = np.asarray(split_weights, np.float32)
    split_biases = np.asarray(split_biases, np.float32)
    leaf_logits = np.asarray(leaf_logits, np.float32)
    tree_weights = np.asarray(tree_weights, np.float32)
    lt = float(np.asarray(log_temperature, np.float32).reshape(-1)[0])

    has_bias = bool(np.any(split_biases != 0.0))
    unit_temp = (lt == 0.0)
    f8 = ml_dtypes.float8_e4m3 if A_FP8 else ml_dtypes.bfloat16

    # ---- host layout prep ----
    # Node permutation: within each 64-col tree block, col 0 is padding and
    # level d occupies cols [2^d, 2^(d+1)) holding BFS node (2^d-1)+bitrev_d(r)
    # at col 2^d + r; leaves end up in LSB-first path order = bitrev6(BFS).
    def bitrev(v, bits):
        r = 0
        for _ in range(bits):
            r = (r << 1) | (v & 1)
            v >>= 1
        return r

    node_src = np.zeros(NIP, np.int64)  # padded col -> BFS node (col 0 -> pad)
    for d in range(DEPTH):
        for r in range(2 ** d):
            node_src[2 ** d + r] = (2 ** d - 1) + bitrev(r, d)
    leaf_src = np.array([bitrev(j, DEPTH) for j in range(L)], np.int64)

    # W^T [D, TIP]: permuted + padded node columns, fp8
    wpad = np.zeros((T, NIP, D), np.float32)
    wpad[:, 1:, :] = split_weights[:, node_src[1:], :]
    wT = np.ascontiguousarray(wpad.reshape(TIP, D).T.astype(f8))
    # x^T shards [D, BS] per core, fp8
    xT = x.T.astype(f8)
    xT_shards = [np.ascontiguousarray(xT[:, c * BS:(c + 1) * BS])
                 for c in range(NCORES)]
    # leaf logits: bitrev leaf order, then [TL, C] -> [128, KB, C] with
    # ll3[p, s, :] = permuted row s*128+p
    ll_perm = leaf_logits[:, leaf_src, :].reshape(TL, C)
    ll = np.ascontiguousarray(
        ll_perm.reshape(KB, 128, C).transpose(1, 0, 2).astype(f8))
    # tree-weight softmax (32 scalars on host); wm[p, s] = w_t * T * 1024
    # (the 1/Z completes the muT scale; 1/(T*64*1024) lands at evac)
    twf = tree_weights - tree_weights.max()
    w = np.exp(twf) / np.exp(twf).sum()
    wmz = (w * T * 1024.0).astype(np.float32)
    p_idx = np.arange(128)[:, None]
    s_idx = np.arange(KB)[None, :]
    wm = np.ascontiguousarray(wmz[(s_idx * 128 + p_idx) // 64])

    in_map_common = {"wT": wT, "ll": ll, "wm": wm}
    if has_bias:
        bpad = np.zeros((T, NIP), np.float32)
        bpad[:, 1:] = split_biases[:, node_src[1:]]
        in_map_common["biasb"] = np.ascontiguousarray(
            np.broadcast_to(bpad.reshape(1, TIP), (128, TIP)).astype(np.float32))
    if not unit_temp:
        in_map_common["lt"] = np.full((1, 1), lt, np.float32)

    nc = _get_nc((has_bias, unit_temp, A_FP8))
    in_maps = [{"xT": xT_shards[c], **in_map_common} for c in range(NCORES)]
    try:
        res = run_bass_kernel_spmd(nc, in_maps, core_ids=list(range(NCORES)))
    except ModuleNotFoundError:
        # BASS_TRACE set but the axon NTFF hook isn't shipped in this
        # container; retry without tracing.
        os.environ["BASS_NEVER_TRACE"] = "1"
        res = run_bass_kernel_spmd(nc, in_maps, core_ids=list(range(NCORES)))
    global LAST_RESULT
    LAST_RESULT = res
    out = np.concatenate([np.asarray(r["out"]).astype(np.float32)
                          for r in res.results], axis=0)
    return np.ascontiguousarray(out)


LAST_RESULT = None
